# revision 5
# baseline (speedup 1.0000x reference)
"""Trainium2 Bass kernel: masked multi-head attention, sharded across 8 NeuronCores.

Problem shapes (hardcoded): B=2, T=2048, D=1024, H=16 heads, dh=64.

Sharding: one SPMD program with two phases (one per batch element). In each
phase every core handles 2 of the 16 heads (core c -> heads 2c, 2c+1), so the
16 heads of each batch are spread over all 8 cores. This load-balances the
data-dependent work (Q_len/V_len trim the q/k tile counts per batch).

All matmul operands are fp16 (inputs cast on host): fp32 matmuls cost 4
cycles/row on the TRN2 PE vs 1 for fp16, and fp16 halves the input DMA bytes.
PSUM accumulation stays fp32, so the error vs the fp32 reference is ~1e-3.

The first phase's attention is split into kv-chunk PASSES (kt 0..3, then
kt 4..NK-1): each pass accumulates a partial softmax numerator + denominator
(no exp rescaling needed -- exp(S) can't overflow fp32 here) and the host sums
the partials. This lets attention start after only the first 512-key chunk of
K/V has been DMA'd instead of all of it, overlapping the input stream with
compute. Output DMAs are deferred behind the next pass's input dma_starts so
the (in-order) sync queue never stalls input prefetch on compute.

Device algorithm per pass, per 512-wide q chunk, per pair of key tiles:
      S^T[kt] = kT_tile.T @ qT_chunk          (PE, K=64, heads row-packed)
      E = exp(scale*S^T) pair-at-a-time       (ACT -> fp16, amortizes the
                                               ~185ns/instr ACT access cost)
      [O^T; d] += v_aug.T @ E                 (PE, K=128; psum row 64 = d)
with a skew-1 software pipeline (next pair's S/exp before this pair's PV) and
next-chunk q-projection matmuls interleaved between pairs as PE fillers.
v_aug rows for tokens >= V_len are zero (host zeroes V) and their ones-column
entry is zeroed on device, which replaces the additive -1e12 key mask exactly.
The host does the final divide-by-denominator, query mask and transpose.
"""

import math
import os
from contextlib import ExitStack

import numpy as np

import concourse.bacc as bacc
import concourse.mybir as mybir
import concourse.tile as tile
from concourse.bass_utils import run_bass_kernel_spmd

F32 = mybir.dt.float32
F16 = mybir.dt.float16
EXP = mybir.ActivationFunctionType.Exp
XNP = np.float16

B, T, D, H, DH = 2, 2048, 1024, 16, 64
N_CORES = 8
KCH = D // 128          # 8 contraction chunks of the model dim
SCALE = 1.0 / math.sqrt(DH)

LAST_EXEC_NS = None     # filled when BASS_TRACE=1


def _ensure_ntff_hook():
    """run_bass_kernel_spmd(trace=True) imports antenv.axon_hooks, which some
    containers lack; synthesize it (backed by libaxon_pjrt's NRT profiling)
    so tracing degrades gracefully instead of crashing."""
    import sys
    import types
    try:
        import antenv.axon_hooks  # noqa: F401
        return
    except ImportError:
        pass
    try:
        import antenv
        from trn_agent_boot.trn_boot import _ntff_profile_via_ctypes
        hook = _ntff_profile_via_ctypes("/opt/axon/libaxon_pjrt.so")
    except Exception:
        antenv = None
        hook = None
    try:
        m = types.ModuleType("antenv.axon_hooks")
        m._hook = hook
        m.set_axon_ntff_profile_hook = lambda h: setattr(m, "_hook", h)
        m.get_axon_ntff_profile_hook = lambda: m._hook
        sys.modules["antenv.axon_hooks"] = m
        if antenv is not None:
            antenv.axon_hooks = m
    except Exception:
        pass


def _ceil_div(a, b):
    return -(-a // b)


def _emit_phase(nc, tc, P, ph):
    """Emit one batch element's phase into the program."""
    s = str(ph["b"])
    io = ph["io"]
    NK, Qp, Kp = ph["NK"], ph["Qp"], ph["Kp"]
    scale, vrem = ph["scale"], ph["vrem"]
    wts = P["wts"]
    KC = _ceil_div(Kp, 512)
    NQC = _ceil_div(Qp, 512)
    kcs = [None] * KC
    vas = [None] * NK

    def kvproj(c):
        """Project key chunk c into kT (kcs[c]) and v_aug tiles (vas)."""
        n = min(512, Kp - c * 512)
        xt = P["x"].tile([128, KCH, n], F16, tag="xt", name="xt", bufs=3)
        for k in range(KCH):
            # per-slice DMAs: k-proj matmul k starts as soon as slice k
            # lands, and 8 concurrent dma_starts spread across DMA queues
            nc.sync.dma_start(xt[:, k, :], io["xk"][:, k, c * 512:c * 512 + n])
        ps = P["pp"].tile([128, n], F32, tag="pp", name="pp")
        for k in range(KCH):
            nc.tensor.matmul(ps[:], lhsT=wts["wk"][:, k, :], rhs=xt[:, k, :],
                             start=(k == 0), stop=(k == KCH - 1))
        kc = P["persist"].tile([128, n], F16, tag="kT" + s, name="kT" + s,
                               bufs=KC)
        nc.vector.tensor_copy(kc[:], ps[:])
        kcs[c] = kc

        ng = n // 128
        xtv = P["x"].tile([128, ng, KCH, 128], F16, tag="xtv", name="xtv",
                          bufs=2, padded_shape=[128, 4, KCH, 128])
        for j in range(ng):
            nc.sync.dma_start(xtv[:, j], io["xv"][:, c * 4 + j])
        for j in range(ng):
            kt = c * 4 + j
            va = P["persist"].tile([128, 2, 65], F16, tag="va" + s,
                                   name="va" + s, bufs=NK)
            if kt == NK - 1 and vrem is not None:
                # partial last key tile: ones only on the valid rows, so
                # padded keys add nothing to the softmax denominator
                nc.vector.memset(va[:, :, 64:65], 0.0)
                nc.vector.memset(va[0:vrem, :, 64:65], 1.0)
            else:
                nc.vector.memset(va[:, :, 64:65], 1.0)
            ps2 = P["pp"].tile([128, 128], F32, tag="pp", name="ps2")
            for k in range(KCH):
                nc.tensor.matmul(ps2[:], lhsT=xtv[:, j, k, :],
                                 rhs=wts["wv"][:, k, :],
                                 start=(k == 0), stop=(k == KCH - 1))
            nc.vector.tensor_copy(va[:, :, 0:64],
                                  ps2[:].rearrange("p (g d) -> p g d", g=2))
            vas[kt] = va

    def emit_qproj(c):
        """Returns (qc, thunks): DMA issues now, matmuls run via thunks."""
        n = min(512, Qp - c * 512)
        xtq = P["x"].tile([128, KCH, n], F16, tag="xtq", name="xtq", bufs=2)
        if ph.get("first") and c == 0:
            for k in range(KCH):
                nc.sync.dma_start(xtq[:, k, :], io["xq"][:, k, 0:n])
        else:
            nc.sync.dma_start(xtq[:], io["xq"][:, :, c * 512:c * 512 + n])
        ps = P["pp"].tile([128, n], F32, tag="pp", name="psq")
        qc = P["persist"].tile([128, n], F16, tag="qT" + s, name="qT" + s,
                               bufs=NQC)

        def mk(k):
            def go():
                nc.tensor.matmul(ps[:], lhsT=wts["wq"][:, k, :],
                                 rhs=xtq[:, k, :],
                                 start=(k == 0), stop=(k == KCH - 1),
                                 skip_group_check=True)
            return go

        thunks = [mk(k) for k in range(KCH)]
        thunks.append(lambda: nc.vector.tensor_copy(qc[:], ps[:]))
        return qc, thunks

    deferred = ph["deferred"]   # out-DMA thunks, flushed behind input DMAs

    def attention(c, kts, out_d, qc, fill):
        """One q chunk of one pass: S/exp/PV over key tiles `kts`."""
        n = min(512, Qp - c * 512)
        groups = [kts[j:j + 2] for j in range(0, len(kts), 2)]
        NG = len(groups)
        otd = [P["ot"].tile([65, n], F32, tag="ot", name="otd") for _ in (0, 1)]
        per_g = _ceil_div(len(fill), NG) if fill else 0

        def emit_sg(gi):
            g = groups[gi]
            es = []
            for h in (0, 1):
                sps = P["sp"].tile([128, len(g), n], F32, tag="sp", name="sps")
                for i, kt in enumerate(g):
                    nc.tensor.matmul(
                        sps[:, i, :],
                        lhsT=kcs[kt // 4][h * 64:(h + 1) * 64,
                                          (kt % 4) * 128:(kt % 4) * 128 + 128],
                        rhs=qc[h * 64:(h + 1) * 64, :],
                        start=True, stop=True)
                e = P["e"].tile([128, len(g), n], F16, tag="e", name="e")
                nc.scalar.activation(e[:], sps[:], EXP, scale=scale)
                es.append(e)
            return es

        es_prev = emit_sg(0)
        for gi in range(NG):
            es_cur = es_prev
            if gi + 1 < NG:
                es_prev = emit_sg(gi + 1)
            for t in fill[:per_g]:
                t()
            fill = fill[per_g:]
            for h in (0, 1):
                for i, kt in enumerate(groups[gi]):
                    nc.tensor.matmul(otd[h][:], lhsT=vas[kt][:, h, :],
                                     rhs=es_cur[h][:, i, :],
                                     start=(kt == kts[0]), stop=(kt == kts[-1]),
                                     skip_group_check=True)
        for t in fill:
            t()
        for h in (0, 1):
            ob = P["ob"].tile([65, n], F32, tag="ob", name="ob", bufs=10)
            nc.vector.tensor_copy(ob[:], otd[h][:])
            deferred.append(lambda h=h, ob=ob, n=n, c=c: nc.sync.dma_start(
                out_d[h][:, c * 512:c * 512 + n], ob[:]))

    if ph.get("first") and KC > 1:
        passes = [list(range(0, min(4, NK))), list(range(4, NK))]
    else:
        passes = [list(range(NK))]
    ph["npass"] = len(passes)

    kvproj(0)
    if len(passes) == 1:
        for c in range(1, KC):
            kvproj(c)
    qc0, th = emit_qproj(0)
    for t in th:
        t()
    qcs = {0: qc0}

    for pi, kts in enumerate(passes):
        if pi > 0:
            for c in range(1, KC):
                kvproj(c)
            # flush previous pass's output DMAs only after the next pass's
            # input dma_starts are queued (sync queue is in-order)
            for t in deferred:
                t()
            deferred.clear()
        out_d = io["out"][pi]
        for c in range(NQC):
            if pi == 0 and c + 1 < NQC:
                qcs[c + 1], fill = emit_qproj(c + 1)
            else:
                fill = []
            attention(c, kts, out_d, qcs[c], fill)


def _build_program(phases):
    nc = bacc.Bacc("TRN2", target_bir_lowering=False, debug=False,
                   num_devices=N_CORES)
    for ph in phases:
        s = str(ph["b"])
        Qp, Kp, NK = ph["Qp"], ph["Kp"], ph["NK"]
        npass = 2 if (ph.get("first") and Kp > 512) else 1
        io = {
            "xq": nc.dram_tensor("xq" + s, [128, KCH, Qp], F16, kind="ExternalInput"),
            "xk": nc.dram_tensor("xk" + s, [128, KCH, Kp], F16, kind="ExternalInput"),
            "xv": nc.dram_tensor("xv" + s, [128, NK, KCH, 128], F16, kind="ExternalInput"),
            "out": [nc.dram_tensor(f"out{s}_{p}", [2, 65, Qp], F32,
                                   kind="ExternalOutput") for p in range(npass)],
        }
        ph["io"] = io

    with tile.TileContext(nc) as tc, ExitStack() as ctx:
        P = {
            "w": ctx.enter_context(tc.tile_pool(name="w", bufs=1)),
            "x": ctx.enter_context(tc.tile_pool(name="x", bufs=3)),
            "e": ctx.enter_context(tc.tile_pool(name="e", bufs=6)),
            "ob": ctx.enter_context(tc.tile_pool(name="ob", bufs=10)),
            "persist": ctx.enter_context(tc.tile_pool(name="persist", bufs=1)),
            "pp": ctx.enter_context(tc.tile_pool(name="pp", bufs=2, space="PSUM")),
            "sp": ctx.enter_context(tc.tile_pool(name="sp", bufs=2, space="PSUM")),
            "ot": ctx.enter_context(tc.tile_pool(name="ot", bufs=2, space="PSUM")),
        }
        # prime the ACT exp table while the first DMAs are in flight
        warm = P["w"].tile([1, 1], F32, tag="actwarm", name="actwarm")
        nc.vector.memset(warm[:], 0.0)
        nc.scalar.activation(warm[:], warm[:], EXP)
        wts = {}
        for nm in ("wk", "wv", "wq"):   # k-proj runs first: load wk first
            wd = nc.dram_tensor(nm, [128, KCH, 128], F16, kind="ExternalInput")
            t = P["w"].tile([128, KCH, 128], F16, tag=nm, name=nm)
            nc.sync.dma_start(t[:], wd[:])
            wts[nm] = t
        P["wts"] = wts
        deferred = []
        for ph in phases:
            ph["deferred"] = deferred
            _emit_phase(nc, tc, P, ph)
        for t in deferred:
            t()
    nc.compile()
    return nc


def _prep_xT(X, P):
    """[T, D] -> [128, KCH, P] with x[p, k, t] = X[t, k*128 + p]."""
    Xp = np.ascontiguousarray(X[:P].T)                 # [D, P]
    return np.ascontiguousarray(
        Xp.reshape(KCH, 128, P).transpose(1, 0, 2)).astype(XNP)  # [128, KCH, P]


def _prep_xv(X, P):
    """[T, D] -> [128, P//128, KCH, 128] token-group-major (2KB-contiguous
    per partition per group, so per-group sub-DMAs run at full DMA speed)."""
    G = P // 128
    Xg = X[:P].reshape(G, 128, KCH, 128)               # [g, t, k, p]
    return np.ascontiguousarray(Xg.transpose(3, 0, 2, 1)).astype(XNP)


def _prep_w(W, c):
    """[D, H*DH] -> per-core [128, KCH, 128] slice of heads (2c, 2c+1)."""
    Ws = W[:, c * 128:(c + 1) * 128]                   # [D, 128]
    return np.ascontiguousarray(
        Ws.reshape(KCH, 128, 128).transpose(1, 0, 2)).astype(XNP)


def kernel(Q_seq, K_seq, V_seq, Q_len, V_len, WQ, WK, WV):
    global LAST_EXEC_NS
    Q_seq = np.asarray(Q_seq, dtype=np.float32)
    K_seq = np.asarray(K_seq, dtype=np.float32)
    V_seq = np.asarray(V_seq, dtype=np.float32)
    WQ = np.asarray(WQ, dtype=np.float32)
    WK = np.asarray(WK, dtype=np.float32)
    WV = np.asarray(WV, dtype=np.float32)
    qlen = [int(np.asarray(Q_len)[b, 0]) for b in range(B)]
    vlen = [int(np.asarray(V_len)[b, 0]) for b in range(B)]

    phases = []
    for b in range(B):
        Qp = _ceil_div(qlen[b], 32) * 32   # q only needs 32-elem alignment
        if Qp == 0:
            continue  # whole batch output is zero
        if vlen[b] > 0:
            NK, scale = _ceil_div(vlen[b], 128), SCALE
            vrem = vlen[b] - (NK - 1) * 128
            if vrem == 128:
                vrem = None
        else:
            # all keys masked -> reference softmax degenerates to uniform
            # over all T keys; exp(0*S) = 1 reproduces it exactly.
            NK, scale, vrem = T // 128, 0.0, None
        phases.append(dict(b=b, NK=NK, Qp=Qp, Kp=NK * 128, scale=scale,
                           vrem=vrem, first=not phases))

    out = np.zeros((B, T, H * DH), dtype=np.float32)
    if not phases:
        return out

    nc = _build_program(phases)

    # per-phase data shared by all cores
    shared = {}
    for ph in phases:
        b, s, Kp = ph["b"], str(ph["b"]), ph["Kp"]
        Vb = V_seq[b]
        if 0 < vlen[b] < Kp:
            Vb = Vb.copy()
            Vb[vlen[b]:Kp] = 0.0   # padded keys: zero v rows -> no output term
        shared[s] = {
            "xq" + s: _prep_xT(Q_seq[b], ph["Qp"]),
            "xk" + s: _prep_xT(K_seq[b], Kp),
            "xv" + s: _prep_xv(Vb, Kp),
        }

    in_maps = []
    for c in range(N_CORES):
        m = {}
        for ph in phases:
            m.update(shared[str(ph["b"])])
        m["wq"] = _prep_w(WQ, c)
        m["wk"] = _prep_w(WK, c)
        m["wv"] = _prep_w(WV, c)
        in_maps.append(m)

    trace = bool(os.environ.get("BASS_TRACE"))
    if trace:
        _ensure_ntff_hook()
    res = run_bass_kernel_spmd(nc, in_maps, list(range(N_CORES)), trace=trace)
    LAST_EXEC_NS = res.exec_time_ns

    for c in range(N_CORES):
        r = res.results[c]
        for ph in phases:
            b, s = ph["b"], str(ph["b"])
            ql = qlen[b]
            acc = r["out" + s + "_0"].astype(np.float64)
            for p in range(1, ph["npass"]):
                acc += r[f"out{s}_{p}"]
            for h in (0, 1):
                head = 2 * c + h
                num = acc[h, 0:64, :ql]
                den = acc[h, 64, :ql]
                out[b, :ql, head * DH:(head + 1) * DH] = (num / den).T
    return out
